# revision 31
# baseline (speedup 1.0000x reference)
"""Trainium2 Bass kernel for a dense transformer block (B=2, T=2048, C=1024,
16 heads, causal attention with x64 score scale, MLP 4x), distributed over
8 NeuronCores.

Sharding: token-parallel.  Cores 0-3 take batch element 0, cores 4-7 batch
element 1.  Within a batch element the 16 query tiles of 128 tokens are dealt
round-robin (core j gets tiles j, j+4, j+8, j+12), which balances causal
attention cost and keeps the instruction stream identical across cores (SPMD);
core-dependent causal boundaries are handled with host-computed additive
masks.  K/V are computed redundantly per core for its whole batch element
(no collectives).  K streams through a DRAM scratch (per-head-pair reads are
double-buffered); V stays SBUF-resident channel-major and is transposed
per-head-pair on the PE.

Precision: the attention-score path (x, LN1, Wq/Wk/Wv projections, scores)
runs in float32r (fp32 with ~11-bit mantissa, full PE rate for moving dim >=
256).  The value/output path (attention weights, out-proj, MLP) runs in bf16,
which also halves those weights' HBM traffic and enables fast weight load.
LayerNorm statistics are computed via ones-vector matmuls on the partition
(channel) axis since activations live transposed ([C, T]) on chip.  The x64
score scale is folded into the softmax exp (scale=64), and softmax
normalization is applied to the (small) attention output instead of the
attention matrix.
"""
import numpy as np
import ml_dtypes

import concourse.bass as bass
import concourse.mybir as mybir
import concourse.tile as tile
from concourse.masks import make_identity
from concourse.vector_clock import ScopedClock
from concourse import bass_utils
from concourse.bass_utils import run_bass_kernel_spmd

P = 128
B, T, C = 2, 2048, 1024
NH, HD = 16, 64
NCT = C // P          # 8 channel tiles
NTC = T // 512        # 4 token 512-chunks per batch element
TOWN = 512            # own query tokens per core
NQT = TOWN // P       # 4 own query tiles
NG = NH // 2          # 8 head pairs
LN_EPS = 1e-5
FP = mybir.dt.float32
FR = mybir.dt.float32r
BF = mybir.dt.bfloat16
OP = mybir.AluOpType
AF = mybir.ActivationFunctionType
AX = mybir.AxisListType

# ---------------------------------------------------------------------------
# Workaround for walrus "Too many sync wait commands": most instruction
# structs in this compiler build accept only ~1 sync-wait.  Hoist overflow
# waits onto same-engine NoOps, and split the kernel-tail drain's
# global-clock waits across one drain instruction per clock domain.
# ---------------------------------------------------------------------------
_orig_commit_and_lower = tile.TileContext._commit_and_lower


def _split_commit_and_lower(self, inst, original_block, old_bb_map, bb_to_exit_bb):
    si = getattr(inst, "sync_info", None)
    if (
        si is not None
        and si.on_wait
        and len(si.on_wait) > 1
        and type(inst).__name__.startswith("Inst")
    ):
        waits = list(si.on_wait)
        for w in waits[:-1]:
            nop = mybir.InstNoOp(
                name=self.nc.get_next_instruction_name(),
                sync_info=mybir.SyncInfo(on_wait=[w], on_update=[]),
                bass_nofuse=True,
                engine=inst.engine,
            )
            _orig_commit_and_lower(self, nop, original_block, old_bb_map, bb_to_exit_bb)
        inst.sync_info = mybir.SyncInfo(on_wait=waits[-1:], on_update=list(si.on_update))
    return _orig_commit_and_lower(self, inst, original_block, old_bb_map, bb_to_exit_bb)


def _split_drain_and_barrier(self, tick_clock, wait_clock):
    gc = tick_clock.global_clock
    entries = []
    scoped = gc.items() if hasattr(gc, "items") else [(None, gc)]
    for scope, vc in scoped:
        for proc in range(len(vc)):
            t = vc[proc]
            if t > 0:
                entries.append((scope, proc, t))
    if entries:
        for scope, proc, t in entries:
            drain_inst = self.nc.sync.drain()
            req = ScopedClock()
            req.require_at_least(scope, proc, t)
            wait_clock.add_sem_waits(drain_inst.ins, req)
    else:
        drain_inst = self.nc.sync.drain()
        wait_clock.add_sem_waits(
            drain_inst.ins, ScopedClock({None: tick_clock.global_clock})
        )
    self.nc.all_engine_barrier()
    assert self.sems is not None
    popped = self.nc._tile_sem_poison_stack.pop()
    assert popped is self._sem_poison
    self.nc.clear_and_free_semaphores(list(self.sems.allocated().values()))
    self.nc.all_engine_barrier()


def _apply_tile_patch():
    tile.TileContext._commit_and_lower = _split_commit_and_lower
    tile.TileContext._drain_and_barrier = _split_drain_and_barrier


# ---------------------------------------------------------------------------
# Host-side helpers
# ---------------------------------------------------------------------------

def _r12(a):
    """Round fp32 to float32r's grid (~11 mantissa bits) so on-device fp32r
    consumers see exactly representable values."""
    u = np.ascontiguousarray(a, np.float32).view(np.uint32).astype(np.uint64)
    u = (u + np.uint64(1 << 11)) & np.uint64(0xFFFFF000)
    return (u & np.uint64(0xFFFFFFFF)).astype(np.uint32).view(np.float32)


def _lhsT_tiles(w, km, mm):
    """[K, M] weight -> [M/128, K/128, 128, 128] lhsT tiles (w[m][k] block)."""
    k, m = w.shape
    return np.ascontiguousarray(
        w.reshape(km, P, mm, P).transpose(2, 0, 1, 3)
    )


# ---------------------------------------------------------------------------
# Device kernel builder
# ---------------------------------------------------------------------------

def _build(nc):
    xT = nc.dram_tensor("xT", [C, T], FP, kind="ExternalInput").ap()
    xTo = nc.dram_tensor("xTo", [C, TOWN], FP, kind="ExternalInput").ap()
    wq = nc.dram_tensor("wq", [NCT, NCT, P, P], FP, kind="ExternalInput").ap()
    wk = nc.dram_tensor("wk", [NCT, NCT, P, P], FP, kind="ExternalInput").ap()
    wv = nc.dram_tensor("wv", [NCT, NCT, P, P], FP, kind="ExternalInput").ap()
    wo = nc.dram_tensor("wo", [NCT, NCT, P, P], BF, kind="ExternalInput").ap()
    w1 = nc.dram_tensor("w1", [32, NCT, P, P], BF, kind="ExternalInput").ap()
    w2 = nc.dram_tensor("w2", [NCT, 32, P, P], BF, kind="ExternalInput").ap()
    gb = nc.dram_tensor("gb", [P, NCT, 4], FP, kind="ExternalInput").ap()
    msk = nc.dram_tensor("msk", [NQT, P, 512], FP, kind="ExternalInput").ap()
    outT = nc.dram_tensor("outT", [C, TOWN], FP, kind="ExternalOutput").ap()
    kscr = nc.dram_tensor("kscr", [C, T], FP, kind="ExternalOutput").ap()

    with tile.TileContext(nc) as tc:
        _build_tc(nc, tc, xT, xTo, wq, wk, wv, wo, w1, w2, gb, msk, outT, kscr)
    return nc


def _ln_chunk(nc, ln_sb, psum_st, src_t, dst_t, sl, g_col, b_col,
              onesP, onesPb, eps_t, nb=2):
    """LayerNorm over the partition(channel) axis of src_t[:, :, sl]
    ([128, NCT, 512] float32r), writing normalized float32r to dst_t (may
    alias src_t).  The stats matmuls use a [P, P] all-(1/C) stationary so
    mean and mean-square land in PSUM already broadcast across partitions,
    avoiding any single-partition [1, 512] work."""
    mean_bc = psum_st.tile([P, 512], FP, tag="mean_bc", bufs=1)
    msq_bc = psum_st.tile([P, 512], FP, tag="msq_bc", bufs=1)
    for ct in range(NCT):
        nc.tensor.matmul(mean_bc[:], onesP[:], src_t[:, ct, sl],
                         start=(ct == 0), stop=(ct == NCT - 1))
    for ct in range(NCT):
        sq = ln_sb.tile([P, 512], BF, tag="sq", bufs=nb)
        nc.scalar.activation(sq[:], src_t[:, ct, sl].bitcast(FP), AF.Square)
        nc.tensor.matmul(msq_bc[:], onesPb[:], sq[:],
                         start=(ct == 0), stop=(ct == NCT - 1))
    mean2 = ln_sb.tile([P, 512], FP, tag="mean2", bufs=1)
    nc.scalar.activation(mean2[:], mean_bc[:], AF.Square)
    var = ln_sb.tile([P, 512], FP, tag="var", bufs=1)
    nc.vector.tensor_tensor(var[:], msq_bc[:], mean2[:], op=OP.subtract)
    sd = ln_sb.tile([P, 512], FP, tag="sd", bufs=1)
    nc.scalar.activation(sd[:], var[:], AF.Sqrt, bias=eps_t[:])
    rb = ln_sb.tile([P, 512], FP, tag="rb", bufs=nb)
    nc.vector.reciprocal(rb[:], sd[:])
    mb = ln_sb.tile([P, 512], FP, tag="mb", bufs=nb)
    nc.vector.tensor_tensor(mb[:], mean_bc[:], rb[:], op=OP.mult)
    for ct in range(NCT):
        t1 = ln_sb.tile([P, 512], FP, tag="t1", bufs=nb)
        nc.vector.tensor_tensor(t1[:], src_t[:, ct, sl], rb[:], op=OP.mult)
        t2 = ln_sb.tile([P, 512], FP, tag="t2", bufs=nb)
        nc.vector.tensor_tensor(t2[:], t1[:], mb[:], op=OP.subtract)
        nc.vector.tensor_scalar(
            dst_t[:, ct, sl], t2[:], g_col[:, ct:ct + 1], b_col[:, ct:ct + 1],
            op0=OP.mult, op1=OP.add,
        )


def _build_tc(nc, tc, xT, xTo, wq, wk, wv, wo, w1, w2, gb, msk, outT, kscr):
    const_cm = tc.tile_pool(name="const", bufs=1)
    const = const_cm.__enter__()
    ident = const.tile([P, P], BF)
    make_identity(nc, ident[:])
    onesP = const.tile([P, P], FR)
    nc.any.memset(onesP[:].bitcast(FP), 1.0 / C)
    onesPb = const.tile([P, P], BF)
    nc.any.memset(onesPb[:], 1.0 / C)
    eps_t = const.tile([P, 1], FP)
    nc.any.memset(eps_t[:], LN_EPS)
    gb_t = const.tile([P, NCT, 4], FP)
    nc.sync.dma_start(gb_t[:], gb)
    mask_t = const.tile([P, NQT, 512], FP)
    nc.sync.dma_start(mask_t[:], msk.rearrange("i p m -> p i m"))

    g1c, b1c = gb_t[:, :, 0], gb_t[:, :, 1]
    g2c, b2c = gb_t[:, :, 2], gb_t[:, :, 3]

    # persistent pools, in stack order of release: persX (xn_own, A -> C),
    # persB (out_t/wo_sb/w1_sb, B -> C; DMAs issued at phase B), pers (v_sb,
    # A -> B, freed first)
    persX_cm = tc.tile_pool(name="persX", bufs=1)
    persX = persX_cm.__enter__()
    xn_own = persX.tile([P, NCT, TOWN], FR)  # 2 MB
    persB_cm = tc.tile_pool(name="persB", bufs=1)
    persB = persB_cm.__enter__()
    out_t = persB.tile([P, NCT, TOWN], BF)    # 1 MB, attention out, B -> C
    wo_sb = persB.tile([P, NCT, NCT, P], BF)  # 2 MB: all of Wo
    pers_cm = tc.tile_pool(name="pers", bufs=1)
    pers = pers_cm.__enter__()
    v_sb = pers.tile([P, NCT, T], BF)        # 4 MB

    # ---------------- Phase A: LN1 + K/V projections ---------------------
    # Single block; LN of half 1 and the own-token LN are emitted between
    # K(half0) and V(half0) so their DVE work hides under PE projections.
    xh_cm = tc.tile_pool(name="xhpool", bufs=2)
    xhpool = xh_cm.__enter__()
    xhs = []
    for half in range(2):
        xh = xhpool.tile([P, NCT, 1024], FR, tag="xh", name=f"xh{half}")
        deng = nc.sync if half == 0 else nc.scalar
        for c2 in range(2):
            sl = slice(c2 * 512, (c2 + 1) * 512)
            gsl = slice(half * 1024 + c2 * 512, half * 1024 + (c2 + 1) * 512)
            deng.dma_start(
                xh[:, :, sl],
                xT[:, gsl].rearrange("(ct p) t -> p ct t", p=P).bitcast(FR))
        xhs.append(xh)

    def _kv_half(nc, half, xh, wdram, wpool, psA, evac, deng):
        for m in range(NCT):
            w_t = wpool.tile([P, NCT, P], FR, tag="wkt", bufs=2, name=f"w{m}")
            deng.dma_start(w_t[:], wdram[m].rearrange("k p m -> p k m").bitcast(FR))
            pss = [psA.tile([P, 512], FP, tag=f"pp{ch}", bufs=1, name=f"ps{ch}")
                   for ch in range(2)]
            for k in range(NCT):
                for ch in range(2):
                    nc.tensor.matmul(pss[ch][:], w_t[:, k],
                                     xh[:, k, ch * 512:(ch + 1) * 512],
                                     start=(k == 0), stop=(k == NCT - 1))
            for ch in range(2):
                evac(m, half * 1024 + ch * 512, pss[ch])

    with tc.tile_pool(name="xo", bufs=1) as xo, \
         tc.tile_pool(name="ln_sb", bufs=1) as ln_sb, \
         tc.tile_pool(name="wpool", bufs=1) as wpool, \
         tc.tile_pool(name="kstage", bufs=2) as kstage, \
         tc.tile_pool(name="psA", bufs=1, space="PSUM") as psA, \
         tc.tile_pool(name="psA_st", bufs=1, space="PSUM") as psA_st:
        xo_t = xo.tile([P, NCT, TOWN], FR)
        nc.sync.dma_start(xo_t[:], xTo.rearrange("(ct p) t -> p ct t", p=P).bitcast(FR))

        def _k_evac(m, off, ps):
            st = kstage.tile([P, 512], FR, tag="kst")
            nc.vector.tensor_copy(st[:], ps[:])
            nc.gpsimd.dma_start(kscr[m * P:(m + 1) * P, off:off + 512],
                                st[:].bitcast(FP))

        def _v_evac(m, off, ps):
            dst = v_sb[:, m, off:off + 512]
            if m % 2 == 0:
                nc.scalar.copy(dst, ps[:])
            else:
                nc.vector.tensor_copy(dst, ps[:])

        for c2 in range(2):
            _ln_chunk(nc, ln_sb, psA_st, xhs[0], xhs[0],
                      slice(c2 * 512, (c2 + 1) * 512),
                      g1c, b1c, onesP, onesPb, eps_t, nb=2)
        _kv_half(nc, 0, xhs[0], wk, wpool, psA, _k_evac, nc.sync)
        for c2 in range(2):
            _ln_chunk(nc, ln_sb, psA_st, xhs[1], xhs[1],
                      slice(c2 * 512, (c2 + 1) * 512),
                      g1c, b1c, onesP, onesPb, eps_t, nb=2)
        _ln_chunk(nc, ln_sb, psA_st, xo_t, xn_own, slice(0, 512),
                  g1c, b1c, onesP, onesPb, eps_t, nb=2)
        _kv_half(nc, 0, xhs[0], wv, wpool, psA, _v_evac, nc.scalar)
        _kv_half(nc, 1, xhs[1], wk, wpool, psA, _k_evac, nc.sync)
        _kv_half(nc, 1, xhs[1], wv, wpool, psA, _v_evac, nc.scalar)

    xh_cm.__exit__(None, None, None)

    # ---------------- Phase B: Q projection + attention ------------------
    # prefetch phase-C weights during attention
    for m in range(NCT):
        nc.scalar.dma_start(wo_sb[:, m], wo[m].rearrange("k p m -> p k m"))

    with tc.tile_pool(name="qpool", bufs=1) as qpool, \
         tc.tile_pool(name="wqpool", bufs=1) as wqpool, \
         tc.tile_pool(name="kvpool", bufs=2) as kvpool, \
         tc.tile_pool(name="vgpool", bufs=2) as vgpool, \
         tc.tile_pool(name="attpool", bufs=2) as attpool, \
         tc.tile_pool(name="attsm", bufs=3) as attsm, \
         tc.tile_pool(name="psB_s", bufs=2, space="PSUM") as psB_s, \
         tc.tile_pool(name="psB_t", bufs=2, space="PSUM") as psB_t, \
         tc.tile_pool(name="psB_o", bufs=1, space="PSUM") as psB_o:
        q_t = qpool.tile([P, NCT, TOWN], FR)  # 2 MB, unscaled q^T (own tokens)
        for m in range(NCT):
            wq_t = wqpool.tile([P, NCT, P], FR, tag="wqt", bufs=2)
            nc.scalar.dma_start(wq_t[:], wq[m].rearrange("k p m -> p k m").bitcast(FR))
            ps = psB_s.tile([P, 512], FP, tag="sps", bufs=3, name="qps")
            for k in range(NCT):
                nc.tensor.matmul(ps[:], wq_t[:, k], xn_own[:, k, :],
                                 start=(k == 0), stop=(k == NCT - 1))
            nc.vector.tensor_copy(q_t[:, m], ps[:])

        for g in range(NG):
            k_g = kvpool.tile([P, T], FR, tag="kg")
            nc.scalar.dma_start(k_g[:], kscr[g * P:(g + 1) * P, :].bitcast(FR))
            # v_g: token-major V for this head pair, via PE transpose
            v_g = vgpool.tile([P, T // P, P], BF, tag="vg")
            for kk in range(NTC):
                ps_v = psB_t.tile([P, 512], BF, tag="tps", bufs=2, name="ps_v")
                for b4 in range(4):
                    blk = kk * 4 + b4
                    nc.tensor.transpose(ps_v[:, b4 * P:(b4 + 1) * P],
                                        v_sb[:, g, blk * P:(blk + 1) * P],
                                        ident[:])
                ev = v_g[:, kk * 4:(kk + 1) * 4, :].rearrange("p n d -> p (n d)")
                nc.scalar.copy(ev, ps_v[:])

            for i in range(NQT):
                nch = i + 1
                scs = []
                mxs = []
                # scores for both head halves first (keeps PE dense while
                # the first half's softmax runs on scalar/vector)
                for h2 in range(2):
                    pb = h2 * 64
                    q_sl = q_t[pb:pb + 64, g, i * P:(i + 1) * P]
                    sc = attpool.tile([P, T], FP, tag="scs", name=f"sc{h2}")
                    mx = attsm.tile([P, NQT], FP, tag="mx", name=f"mx{h2}")
                    for kk in range(nch):
                        ps_s = psB_s.tile([P, 512], FP, tag="sps", bufs=3)
                        nc.tensor.matmul(ps_s[:], q_sl,
                                         k_g[pb:pb + 64, kk * 512:(kk + 1) * 512],
                                         start=True, stop=True)
                        sc_chunk = sc[:, kk * 512:(kk + 1) * 512]
                        if kk == i:
                            nc.vector.tensor_tensor(sc_chunk, ps_s[:],
                                                    mask_t[:, i, :], op=OP.add)
                            nc.vector.tensor_reduce(mx[:, kk:kk + 1], sc_chunk,
                                                    axis=AX.X, op=OP.max)
                        else:
                            nc.scalar.copy(sc_chunk, ps_s[:])
                            nc.vector.tensor_reduce(mx[:, kk:kk + 1], ps_s[:],
                                                    axis=AX.X, op=OP.max)
                    scs.append(sc)
                    mxs.append(mx)
                for h2 in range(2):
                    pb = h2 * 64
                    sc, mx = scs[h2], mxs[h2]
                    nmb = attsm.tile([P, 1], FP, tag="nmb", name=f"nmb{h2}")
                    nc.vector.tensor_reduce(nmb[:], mx[:, 0:nch], axis=AX.X,
                                            op=OP.max)
                    nc.vector.tensor_scalar_mul(nmb[:], nmb[:], -64.0)
                    att = attpool.tile([P, T], BF, tag="att", name=f"att{h2}")
                    den = attsm.tile([P, 1], FP, tag="den", name=f"den{h2}")
                    nc.scalar.activation(
                        att[:, 0:nch * 512], sc[:, 0:nch * 512],
                        AF.Exp, bias=nmb[:], scale=64.0, accum_out=den[:])
                    rden = attsm.tile([P, 1], FP, tag="rden", name=f"rden{h2}")
                    nc.vector.reciprocal(rden[:], den[:])
                    # transpose unnormalized e^(s-m) -> attT [k, q]
                    attT = attpool.tile([P, NQT * 4, P], BF, tag="attT",
                                        name=f"attT{h2}")
                    for kk in range(nch):
                        ps_t = psB_t.tile([P, 512], BF, tag="tps", bufs=2)
                        for b4 in range(4):
                            blk = kk * 4 + b4
                            nc.tensor.transpose(ps_t[:, b4 * P:(b4 + 1) * P],
                                                att[:, blk * P:(blk + 1) * P],
                                                ident[:])
                        ev = attT[:, kk * 4:(kk + 1) * 4, :].rearrange(
                            "p n d -> p (n d)")
                        if kk % 2 == 1:
                            nc.scalar.copy(ev, ps_t[:])
                        else:
                            nc.vector.tensor_copy(ev, ps_t[:])
                    # AV: attT stationary, token-major v moving; out [q, d]
                    ps_o = psB_o.tile([P, 2, 64], FP, tag="ops", bufs=1,
                                      name="ps_o")
                    for blk in range(nch * 4):
                        nc.tensor.matmul(ps_o[:, h2, :], attT[:, blk, :],
                                         v_g[:, blk, pb:pb + 64],
                                         start=(blk == 0),
                                         stop=(blk == nch * 4 - 1))
                    o_sb = attsm.tile([P, 64], BF, tag="osb", name=f"osb{h2}")
                    nc.vector.tensor_scalar_mul(o_sb[:], ps_o[:, h2, :], rden[:])
                    # transpose [q, d] -> [d, q], landing on partitions pb..
                    ps_ot = psB_o.tile([P, 2, P], BF, tag="otps", bufs=1,
                                       name="ps_ot")
                    nc.tensor.transpose(ps_ot[pb:pb + 64, h2, :], o_sb[:], ident[:])
                    if h2 == 0:
                        nc.scalar.copy(out_t[pb:pb + 64, g, i * P:(i + 1) * P],
                                       ps_ot[pb:pb + 64, h2, :])
                    else:
                        nc.vector.tensor_copy(
                            out_t[pb:pb + 64, g, i * P:(i + 1) * P],
                            ps_ot[pb:pb + 64, h2, :])

    pers_cm.__exit__(None, None, None)  # frees v_sb

    # ---------------- Phase C: out-proj, LN2, MLP ------------------------
    with tc.tile_pool(name="wpoolC", bufs=1) as wpoolC, \
         tc.tile_pool(name="ln_sbC", bufs=1) as ln_sbC, \
         tc.tile_pool(name="apool", bufs=1) as apool, \
         tc.tile_pool(name="hpool", bufs=1) as hpool, \
         tc.tile_pool(name="opool", bufs=2) as opool, \
         tc.tile_pool(name="psC", bufs=3, space="PSUM") as psC, \
         tc.tile_pool(name="psC_st", bufs=1, space="PSUM") as psC_st:
        h_t = hpool.tile([P, NCT, TOWN], FR)
        h2f = hpool.tile([P, NCT, TOWN], FR)
        h2b = hpool.tile([P, NCT, TOWN], BF)
        for m in range(NCT):
            ps = psC.tile([P, 512], FP, tag="psC", bufs=3)
            for k in range(NCT):
                nc.tensor.matmul(ps[:], wo_sb[:, m, k, :], out_t[:, k, :],
                                 start=(k == 0), stop=(k == NCT - 1))
            nc.vector.tensor_tensor(h_t[:, m], ps[:], xn_own[:, m], op=OP.add)

        _ln_chunk(nc, ln_sbC, psC_st, h_t, h2f, slice(0, 512),
                  g2c, b2c, onesP, onesPb, eps_t, nb=1)
        for m in range(NCT):
            if m % 2 == 0:
                nc.vector.tensor_copy(h2b[:, m], h2f[:, m])
            else:
                nc.scalar.copy(h2b[:, m], h2f[:, m].bitcast(FP))

        a_t = apool.tile([P, 32, TOWN], BF)    # 4 MB
        for m in range(32):
            w1_t = wpoolC.tile([P, NCT, P], BF, tag="w1t", bufs=3)
            nc.scalar.dma_start(w1_t[:], w1[m].rearrange("k p m -> p k m"))
            w1_sl = w1_t[:]
            ps = psC.tile([P, 512], FP, tag="psC", bufs=3)
            for k in range(NCT):
                nc.tensor.matmul(ps[:], w1_sl[:, k], h2b[:, k, :],
                                 start=(k == 0), stop=(k == NCT - 1))
            if m % 2 == 0:
                nc.scalar.activation(a_t[:, m], ps[:], AF.Relu)
            else:
                nc.vector.tensor_scalar_max(a_t[:, m], ps[:], 0.0)

        for m in range(NCT):
            w2_t = wpoolC.tile([P, 32, P], BF, tag="w2t", bufs=2)
            nc.scalar.dma_start(w2_t[:], w2[m].rearrange("k p m -> p k m"))
            ps = psC.tile([P, 512], FP, tag="psC", bufs=3)
            for k in range(32):
                nc.tensor.matmul(ps[:], w2_t[:, k], a_t[:, k, :],
                                 start=(k == 0), stop=(k == 31))
            o_m = opool.tile([P, 512], FP, tag="om")
            nc.vector.tensor_tensor(o_m[:], ps[:], h2f[:, m], op=OP.add)
            nc.sync.dma_start(outT[m * P:(m + 1) * P, :], o_m[:])

    persB_cm.__exit__(None, None, None)
    persX_cm.__exit__(None, None, None)
    const_cm.__exit__(None, None, None)


# ---------------------------------------------------------------------------
# Public entry point
# ---------------------------------------------------------------------------
_cache = {}


def _get_nc():
    if "nc" not in _cache:
        _apply_tile_patch()
        nc = bass.Bass("TRN2", target_bir_lowering=False, debug=False,
                       num_devices=8)
        _build(nc)
        _cache["nc"] = nc
    return _cache["nc"]


def run(inputs, trace=False):
    x = np.asarray(inputs["x"], np.float32)
    Wk = np.asarray(inputs["Wk"], np.float32)
    Wq = np.asarray(inputs["Wq"], np.float32)
    Wv = np.asarray(inputs["Wv"], np.float32)
    Wo = np.asarray(inputs["Wo"], np.float32)
    W1 = np.asarray(inputs["W1"], np.float32)
    W2 = np.asarray(inputs["W2"], np.float32)
    g1 = np.asarray(inputs["g1"], np.float32)
    b1 = np.asarray(inputs["b1"], np.float32)
    g2 = np.asarray(inputs["g2"], np.float32)
    b2 = np.asarray(inputs["b2"], np.float32)

    bf = ml_dtypes.bfloat16
    wq_t = _r12(_lhsT_tiles(Wq, NCT, NCT))
    wk_t = _r12(_lhsT_tiles(Wk, NCT, NCT))
    wv_t = _r12(_lhsT_tiles(Wv, NCT, NCT))
    wo_t = _lhsT_tiles(Wo, NCT, NCT).astype(bf)
    w1_t = _lhsT_tiles(W1, NCT, 32).astype(bf)
    w2_t = _lhsT_tiles(W2, 32, NCT).astype(bf)
    gbh = np.stack(
        [g1.reshape(NCT, P).T, b1.reshape(NCT, P).T,
         g2.reshape(NCT, P).T, b2.reshape(NCT, P).T], axis=-1
    ).astype(np.float32)  # [P, NCT, 4]

    in_maps = []
    own_tokens_by_core = []
    for c in range(8):
        b = c // 4
        j = c % 4
        tiles = [j + 4 * i for i in range(NQT)]
        toks = np.concatenate([np.arange(t * P, (t + 1) * P) for t in tiles])
        own_tokens_by_core.append((b, toks))
        xT_full = _r12(np.ascontiguousarray(x[b].T))
        xT_own = _r12(np.ascontiguousarray(x[b][toks].T))
        mask = np.zeros((NQT, P, 512), np.float32)
        for i in range(NQT):
            t0 = (j + 4 * i) * P
            Ei = (i + 1) * 512
            cols = (Ei - 512) + np.arange(512)
            rows = t0 + np.arange(P)
            mask[i] = np.where(cols[None, :] <= rows[:, None], 0.0, -1.0e30)
        in_maps.append({
            "xT": xT_full, "xTo": xT_own,
            "wq": wq_t, "wk": wk_t, "wv": wv_t, "wo": wo_t,
            "w1": w1_t, "w2": w2_t, "gb": gbh, "msk": mask,
        })

    nc = _get_nc()
    res = run_bass_kernel_spmd(nc, in_maps, core_ids=list(range(8)),
                               trace=trace)

    out = np.empty((B, T, C), np.float32)
    for c in range(8):
        b, toks = own_tokens_by_core[c]
        out[b, toks, :] = res.results[c]["outT"].T
    return out, res


def kernel(**inputs):
    out, _ = run(inputs, trace=False)
    return out


# revision 33
# speedup vs baseline: 1.0020x; 1.0020x over previous
"""Trainium2 Bass kernel for a dense transformer block (B=2, T=2048, C=1024,
16 heads, causal attention with x64 score scale, MLP 4x), distributed over
8 NeuronCores.

Sharding: token-parallel.  Cores 0-3 take batch element 0, cores 4-7 batch
element 1.  Within a batch element the 16 query tiles of 128 tokens are dealt
round-robin (core j gets tiles j, j+4, j+8, j+12), which balances causal
attention cost and keeps the instruction stream identical across cores (SPMD);
core-dependent causal boundaries are handled with host-computed additive
masks.  K/V are computed redundantly per core for its whole batch element
(no collectives).  K streams through a DRAM scratch (per-head-pair reads are
double-buffered); V stays SBUF-resident channel-major and is transposed
per-head-pair on the PE.

Precision: the attention-score path (x, LN1, Wq/Wk/Wv projections, scores)
runs in float32r (fp32 with ~11-bit mantissa, full PE rate for moving dim >=
256).  The value/output path (attention weights, out-proj, MLP) runs in bf16,
which also halves those weights' HBM traffic and enables fast weight load.
LayerNorm statistics are computed via ones-vector matmuls on the partition
(channel) axis since activations live transposed ([C, T]) on chip.  The x64
score scale is folded into the softmax exp (scale=64), and softmax
normalization is applied to the (small) attention output instead of the
attention matrix.
"""
import numpy as np
import ml_dtypes

import concourse.bass as bass
import concourse.mybir as mybir
import concourse.tile as tile
from concourse.masks import make_identity
from concourse.vector_clock import ScopedClock
from concourse import bass_utils
from concourse.bass_utils import run_bass_kernel_spmd

P = 128
B, T, C = 2, 2048, 1024
NH, HD = 16, 64
NCT = C // P          # 8 channel tiles
NTC = T // 512        # 4 token 512-chunks per batch element
TOWN = 512            # own query tokens per core
NQT = TOWN // P       # 4 own query tiles
NG = NH // 2          # 8 head pairs
LN_EPS = 1e-5
FP = mybir.dt.float32
FR = mybir.dt.float32r
BF = mybir.dt.bfloat16
OP = mybir.AluOpType
AF = mybir.ActivationFunctionType
AX = mybir.AxisListType

# ---------------------------------------------------------------------------
# Workaround for walrus "Too many sync wait commands": most instruction
# structs in this compiler build accept only ~1 sync-wait.  Hoist overflow
# waits onto same-engine NoOps, and split the kernel-tail drain's
# global-clock waits across one drain instruction per clock domain.
# ---------------------------------------------------------------------------
_orig_commit_and_lower = tile.TileContext._commit_and_lower


def _split_commit_and_lower(self, inst, original_block, old_bb_map, bb_to_exit_bb):
    si = getattr(inst, "sync_info", None)
    if (
        si is not None
        and si.on_wait
        and len(si.on_wait) > 1
        and type(inst).__name__.startswith("Inst")
    ):
        waits = list(si.on_wait)
        for w in waits[:-1]:
            nop = mybir.InstNoOp(
                name=self.nc.get_next_instruction_name(),
                sync_info=mybir.SyncInfo(on_wait=[w], on_update=[]),
                bass_nofuse=True,
                engine=inst.engine,
            )
            _orig_commit_and_lower(self, nop, original_block, old_bb_map, bb_to_exit_bb)
        inst.sync_info = mybir.SyncInfo(on_wait=waits[-1:], on_update=list(si.on_update))
    return _orig_commit_and_lower(self, inst, original_block, old_bb_map, bb_to_exit_bb)


def _split_drain_and_barrier(self, tick_clock, wait_clock):
    gc = tick_clock.global_clock
    entries = []
    scoped = gc.items() if hasattr(gc, "items") else [(None, gc)]
    for scope, vc in scoped:
        for proc in range(len(vc)):
            t = vc[proc]
            if t > 0:
                entries.append((scope, proc, t))
    if entries:
        for scope, proc, t in entries:
            drain_inst = self.nc.sync.drain()
            req = ScopedClock()
            req.require_at_least(scope, proc, t)
            wait_clock.add_sem_waits(drain_inst.ins, req)
    else:
        drain_inst = self.nc.sync.drain()
        wait_clock.add_sem_waits(
            drain_inst.ins, ScopedClock({None: tick_clock.global_clock})
        )
    self.nc.all_engine_barrier()
    assert self.sems is not None
    popped = self.nc._tile_sem_poison_stack.pop()
    assert popped is self._sem_poison
    self.nc.clear_and_free_semaphores(list(self.sems.allocated().values()))
    self.nc.all_engine_barrier()


def _apply_tile_patch():
    tile.TileContext._commit_and_lower = _split_commit_and_lower
    tile.TileContext._drain_and_barrier = _split_drain_and_barrier


# ---------------------------------------------------------------------------
# Host-side helpers
# ---------------------------------------------------------------------------

def _r12(a):
    """Round fp32 to float32r's grid (~11 mantissa bits) so on-device fp32r
    consumers see exactly representable values."""
    u = np.ascontiguousarray(a, np.float32).view(np.uint32).astype(np.uint64)
    u = (u + np.uint64(1 << 11)) & np.uint64(0xFFFFF000)
    return (u & np.uint64(0xFFFFFFFF)).astype(np.uint32).view(np.float32)


def _lhsT_tiles(w, km, mm):
    """[K, M] weight -> [M/128, K/128, 128, 128] lhsT tiles (w[m][k] block)."""
    k, m = w.shape
    return np.ascontiguousarray(
        w.reshape(km, P, mm, P).transpose(2, 0, 1, 3)
    )


# ---------------------------------------------------------------------------
# Device kernel builder
# ---------------------------------------------------------------------------

def _build(nc):
    xT = nc.dram_tensor("xT", [C, T], FP, kind="ExternalInput").ap()
    xTo = nc.dram_tensor("xTo", [C, TOWN], FP, kind="ExternalInput").ap()
    wq = nc.dram_tensor("wq", [NCT, NCT, P, P], FP, kind="ExternalInput").ap()
    wk = nc.dram_tensor("wk", [NCT, NCT, P, P], FP, kind="ExternalInput").ap()
    wv = nc.dram_tensor("wv", [NCT, NCT, P, P], FP, kind="ExternalInput").ap()
    wo = nc.dram_tensor("wo", [NCT, NCT, P, P], BF, kind="ExternalInput").ap()
    w1 = nc.dram_tensor("w1", [32, NCT, P, P], BF, kind="ExternalInput").ap()
    w2 = nc.dram_tensor("w2", [NCT, 32, P, P], BF, kind="ExternalInput").ap()
    gb = nc.dram_tensor("gb", [P, NCT, 4], FP, kind="ExternalInput").ap()
    msk = nc.dram_tensor("msk", [NQT, P, 512], FP, kind="ExternalInput").ap()
    outT = nc.dram_tensor("outT", [C, TOWN], FP, kind="ExternalOutput").ap()
    kscr = nc.dram_tensor("kscr", [C, T], FP, kind="ExternalOutput").ap()

    with tile.TileContext(nc) as tc:
        _build_tc(nc, tc, xT, xTo, wq, wk, wv, wo, w1, w2, gb, msk, outT, kscr)
    return nc


def _ln_chunk(nc, ln_sb, psum_st, src_t, dst_t, sl, g_col, b_col,
              onesP, onesPb, eps_t, nb=2):
    """LayerNorm over the partition(channel) axis of src_t[:, :, sl]
    ([128, NCT, 512] float32r), writing normalized float32r to dst_t (may
    alias src_t).  The stats matmuls use a [P, P] all-(1/C) stationary so
    mean and mean-square land in PSUM already broadcast across partitions,
    avoiding any single-partition [1, 512] work."""
    mean_bc = psum_st.tile([P, 512], FP, tag="mean_bc", bufs=1)
    msq_bc = psum_st.tile([P, 512], FP, tag="msq_bc", bufs=1)
    for ct in range(NCT):
        nc.tensor.matmul(mean_bc[:], onesP[:], src_t[:, ct, sl],
                         start=(ct == 0), stop=(ct == NCT - 1))
    for ct in range(NCT):
        sq = ln_sb.tile([P, 512], BF, tag="sq", bufs=nb)
        nc.scalar.activation(sq[:], src_t[:, ct, sl].bitcast(FP), AF.Square)
        nc.tensor.matmul(msq_bc[:], onesPb[:], sq[:],
                         start=(ct == 0), stop=(ct == NCT - 1))
    mean2 = ln_sb.tile([P, 512], FP, tag="mean2", bufs=1)
    nc.scalar.activation(mean2[:], mean_bc[:], AF.Square)
    var = ln_sb.tile([P, 512], FP, tag="var", bufs=1)
    nc.vector.tensor_tensor(var[:], msq_bc[:], mean2[:], op=OP.subtract)
    sd = ln_sb.tile([P, 512], FP, tag="sd", bufs=1)
    nc.scalar.activation(sd[:], var[:], AF.Sqrt, bias=eps_t[:])
    rb = ln_sb.tile([P, 512], FP, tag="rb", bufs=nb)
    nc.vector.reciprocal(rb[:], sd[:])
    mb = ln_sb.tile([P, 512], FP, tag="mb", bufs=nb)
    nc.vector.tensor_tensor(mb[:], mean_bc[:], rb[:], op=OP.mult)
    for ct in range(NCT):
        t1 = ln_sb.tile([P, 512], FP, tag="t1", bufs=nb)
        nc.vector.tensor_tensor(t1[:], src_t[:, ct, sl], rb[:], op=OP.mult)
        t2 = ln_sb.tile([P, 512], FP, tag="t2", bufs=nb)
        nc.vector.tensor_tensor(t2[:], t1[:], mb[:], op=OP.subtract)
        nc.vector.tensor_scalar(
            dst_t[:, ct, sl], t2[:], g_col[:, ct:ct + 1], b_col[:, ct:ct + 1],
            op0=OP.mult, op1=OP.add,
        )


def _build_tc(nc, tc, xT, xTo, wq, wk, wv, wo, w1, w2, gb, msk, outT, kscr):
    const_cm = tc.tile_pool(name="const", bufs=1)
    const = const_cm.__enter__()
    ident = const.tile([P, P], BF)
    make_identity(nc, ident[:])
    onesP = const.tile([P, P], FR)
    nc.any.memset(onesP[:].bitcast(FP), 1.0 / C)
    onesPb = const.tile([P, P], BF)
    nc.any.memset(onesPb[:], 1.0 / C)
    eps_t = const.tile([P, 1], FP)
    nc.any.memset(eps_t[:], LN_EPS)
    gb_t = const.tile([P, NCT, 4], FP)
    nc.sync.dma_start(gb_t[:], gb)
    mask_t = const.tile([P, NQT, 512], FP)
    nc.sync.dma_start(mask_t[:], msk.rearrange("i p m -> p i m"))

    g1c, b1c = gb_t[:, :, 0], gb_t[:, :, 1]
    g2c, b2c = gb_t[:, :, 2], gb_t[:, :, 3]

    # persistent pools, in stack order of release: persX (xn_own, A -> C),
    # persB (out_t/wo_sb/w1_sb, B -> C; DMAs issued at phase B), pers (v_sb,
    # A -> B, freed first)
    persX_cm = tc.tile_pool(name="persX", bufs=1)
    persX = persX_cm.__enter__()
    xn_own = persX.tile([P, NCT, TOWN], FR)  # 2 MB
    persB_cm = tc.tile_pool(name="persB", bufs=1)
    persB = persB_cm.__enter__()
    out_t = persB.tile([P, NCT, TOWN], BF)    # 1 MB, attention out, B -> C
    wo_sb = persB.tile([P, NCT, NCT, P], BF)  # 2 MB: all of Wo
    pers_cm = tc.tile_pool(name="pers", bufs=1)
    pers = pers_cm.__enter__()
    v_sb = pers.tile([P, NCT, T], BF)        # 4 MB

    # ---------------- Phase A: LN1 + K/V projections ---------------------
    # Single block; LN of half 1 and the own-token LN are emitted between
    # K(half0) and V(half0) so their DVE work hides under PE projections.
    xh_cm = tc.tile_pool(name="xhpool", bufs=2)
    xhpool = xh_cm.__enter__()
    xhs = []
    for half in range(2):
        xh = xhpool.tile([P, NCT, 1024], FR, tag="xh", name=f"xh{half}")
        for c2 in range(2):
            sl = slice(c2 * 512, (c2 + 1) * 512)
            gsl = slice(half * 1024 + c2 * 512, half * 1024 + (c2 + 1) * 512)
            nc.sync.dma_start(
                xh[:, :, sl],
                xT[:, gsl].rearrange("(ct p) t -> p ct t", p=P).bitcast(FR))
        xhs.append(xh)

    def _kv_half(nc, half, xh, wdram, wpool, psA, evac, deng):
        for m in range(NCT):
            w_t = wpool.tile([P, NCT, P], FR, tag="wkt", bufs=2, name=f"w{m}")
            deng.dma_start(w_t[:], wdram[m].rearrange("k p m -> p k m").bitcast(FR))
            pss = [psA.tile([P, 512], FP, tag=f"pp{ch}", bufs=1, name=f"ps{ch}")
                   for ch in range(2)]
            for k in range(NCT):
                for ch in range(2):
                    nc.tensor.matmul(pss[ch][:], w_t[:, k],
                                     xh[:, k, ch * 512:(ch + 1) * 512],
                                     start=(k == 0), stop=(k == NCT - 1))
            for ch in range(2):
                evac(m, half * 1024 + ch * 512, pss[ch])

    with tc.tile_pool(name="xo", bufs=1) as xo, \
         tc.tile_pool(name="ln_sb", bufs=1) as ln_sb, \
         tc.tile_pool(name="wpool", bufs=1) as wpool, \
         tc.tile_pool(name="kstage", bufs=2) as kstage, \
         tc.tile_pool(name="psA", bufs=1, space="PSUM") as psA, \
         tc.tile_pool(name="psA_st", bufs=1, space="PSUM") as psA_st:
        xo_t = xo.tile([P, NCT, TOWN], FR)
        nc.sync.dma_start(xo_t[:], xTo.rearrange("(ct p) t -> p ct t", p=P).bitcast(FR))

        def _k_evac(m, off, ps):
            st = kstage.tile([P, 512], FR, tag="kst")
            nc.vector.tensor_copy(st[:], ps[:])
            nc.sync.dma_start(kscr[m * P:(m + 1) * P, off:off + 512],
                              st[:].bitcast(FP))

        def _v_evac(m, off, ps):
            dst = v_sb[:, m, off:off + 512]
            if m % 2 == 0:
                nc.scalar.copy(dst, ps[:])
            else:
                nc.vector.tensor_copy(dst, ps[:])

        for c2 in range(2):
            _ln_chunk(nc, ln_sb, psA_st, xhs[0], xhs[0],
                      slice(c2 * 512, (c2 + 1) * 512),
                      g1c, b1c, onesP, onesPb, eps_t, nb=2)
        _kv_half(nc, 0, xhs[0], wk, wpool, psA, _k_evac, nc.sync)
        for c2 in range(2):
            _ln_chunk(nc, ln_sb, psA_st, xhs[1], xhs[1],
                      slice(c2 * 512, (c2 + 1) * 512),
                      g1c, b1c, onesP, onesPb, eps_t, nb=2)
        _ln_chunk(nc, ln_sb, psA_st, xo_t, xn_own, slice(0, 512),
                  g1c, b1c, onesP, onesPb, eps_t, nb=2)
        _kv_half(nc, 0, xhs[0], wv, wpool, psA, _v_evac, nc.scalar)
        _kv_half(nc, 1, xhs[1], wk, wpool, psA, _k_evac, nc.sync)
        _kv_half(nc, 1, xhs[1], wv, wpool, psA, _v_evac, nc.scalar)

    xh_cm.__exit__(None, None, None)

    # ---------------- Phase B: Q projection + attention ------------------
    # prefetch phase-C weights during attention
    for m in range(NCT):
        nc.scalar.dma_start(wo_sb[:, m], wo[m].rearrange("k p m -> p k m"))

    with tc.tile_pool(name="qpool", bufs=1) as qpool, \
         tc.tile_pool(name="wqpool", bufs=1) as wqpool, \
         tc.tile_pool(name="kvpool", bufs=2) as kvpool, \
         tc.tile_pool(name="vgpool", bufs=2) as vgpool, \
         tc.tile_pool(name="attpool", bufs=2) as attpool, \
         tc.tile_pool(name="attsm", bufs=3) as attsm, \
         tc.tile_pool(name="psB_s", bufs=2, space="PSUM") as psB_s, \
         tc.tile_pool(name="psB_t", bufs=2, space="PSUM") as psB_t, \
         tc.tile_pool(name="psB_o", bufs=1, space="PSUM") as psB_o:
        q_t = qpool.tile([P, NCT, TOWN], FR)  # 2 MB, unscaled q^T (own tokens)
        for m in range(NCT):
            wq_t = wqpool.tile([P, NCT, P], FR, tag="wqt", bufs=2)
            nc.scalar.dma_start(wq_t[:], wq[m].rearrange("k p m -> p k m").bitcast(FR))
            ps = psB_s.tile([P, 512], FP, tag="sps", bufs=3, name="qps")
            for k in range(NCT):
                nc.tensor.matmul(ps[:], wq_t[:, k], xn_own[:, k, :],
                                 start=(k == 0), stop=(k == NCT - 1))
            nc.vector.tensor_copy(q_t[:, m], ps[:])

        for g in range(NG):
            k_g = kvpool.tile([P, T], FR, tag="kg")
            nc.scalar.dma_start(k_g[:], kscr[g * P:(g + 1) * P, :].bitcast(FR))
            # v_g: token-major V for this head pair, via PE transpose
            v_g = vgpool.tile([P, T // P, P], BF, tag="vg")
            for kk in range(NTC):
                ps_v = psB_t.tile([P, 512], BF, tag="tps", bufs=2, name="ps_v")
                for b4 in range(4):
                    blk = kk * 4 + b4
                    nc.tensor.transpose(ps_v[:, b4 * P:(b4 + 1) * P],
                                        v_sb[:, g, blk * P:(blk + 1) * P],
                                        ident[:])
                ev = v_g[:, kk * 4:(kk + 1) * 4, :].rearrange("p n d -> p (n d)")
                nc.scalar.copy(ev, ps_v[:])

            for i in range(NQT):
                nch = i + 1
                scs = []
                mxs = []
                # scores for both head halves first (keeps PE dense while
                # the first half's softmax runs on scalar/vector)
                for h2 in range(2):
                    pb = h2 * 64
                    q_sl = q_t[pb:pb + 64, g, i * P:(i + 1) * P]
                    sc = attpool.tile([P, T], FP, tag="scs", name=f"sc{h2}")
                    mx = attsm.tile([P, NQT], FP, tag="mx", name=f"mx{h2}")
                    for kk in range(nch):
                        ps_s = psB_s.tile([P, 512], FP, tag="sps", bufs=3)
                        nc.tensor.matmul(ps_s[:], q_sl,
                                         k_g[pb:pb + 64, kk * 512:(kk + 1) * 512],
                                         start=True, stop=True)
                        sc_chunk = sc[:, kk * 512:(kk + 1) * 512]
                        if kk == i:
                            nc.vector.tensor_tensor(sc_chunk, ps_s[:],
                                                    mask_t[:, i, :], op=OP.add)
                            nc.vector.tensor_reduce(mx[:, kk:kk + 1], sc_chunk,
                                                    axis=AX.X, op=OP.max)
                        else:
                            nc.scalar.copy(sc_chunk, ps_s[:])
                            nc.vector.tensor_reduce(mx[:, kk:kk + 1], ps_s[:],
                                                    axis=AX.X, op=OP.max)
                    scs.append(sc)
                    mxs.append(mx)
                for h2 in range(2):
                    pb = h2 * 64
                    sc, mx = scs[h2], mxs[h2]
                    nmb = attsm.tile([P, 1], FP, tag="nmb", name=f"nmb{h2}")
                    nc.vector.tensor_reduce(nmb[:], mx[:, 0:nch], axis=AX.X,
                                            op=OP.max)
                    nc.vector.tensor_scalar_mul(nmb[:], nmb[:], -64.0)
                    att = attpool.tile([P, T], BF, tag="att", name=f"att{h2}")
                    den = attsm.tile([P, 1], FP, tag="den", name=f"den{h2}")
                    nc.scalar.activation(
                        att[:, 0:nch * 512], sc[:, 0:nch * 512],
                        AF.Exp, bias=nmb[:], scale=64.0, accum_out=den[:])
                    rden = attsm.tile([P, 1], FP, tag="rden", name=f"rden{h2}")
                    nc.vector.reciprocal(rden[:], den[:])
                    # transpose unnormalized e^(s-m) -> attT [k, q]
                    attT = attpool.tile([P, NQT * 4, P], BF, tag="attT",
                                        name=f"attT{h2}")
                    for kk in range(nch):
                        ps_t = psB_t.tile([P, 512], BF, tag="tps", bufs=2)
                        for b4 in range(4):
                            blk = kk * 4 + b4
                            nc.tensor.transpose(ps_t[:, b4 * P:(b4 + 1) * P],
                                                att[:, blk * P:(blk + 1) * P],
                                                ident[:])
                        ev = attT[:, kk * 4:(kk + 1) * 4, :].rearrange(
                            "p n d -> p (n d)")
                        if kk % 2 == 1:
                            nc.scalar.copy(ev, ps_t[:])
                        else:
                            nc.vector.tensor_copy(ev, ps_t[:])
                    # AV: attT stationary, token-major v moving; out [q, d]
                    ps_o = psB_o.tile([P, 2, 64], FP, tag="ops", bufs=1,
                                      name="ps_o")
                    for blk in range(nch * 4):
                        nc.tensor.matmul(ps_o[:, h2, :], attT[:, blk, :],
                                         v_g[:, blk, pb:pb + 64],
                                         start=(blk == 0),
                                         stop=(blk == nch * 4 - 1))
                    o_sb = attsm.tile([P, 64], BF, tag="osb", name=f"osb{h2}")
                    nc.vector.tensor_scalar_mul(o_sb[:], ps_o[:, h2, :], rden[:])
                    # transpose [q, d] -> [d, q], landing on partitions pb..
                    ps_ot = psB_o.tile([P, 2, P], BF, tag="otps", bufs=1,
                                       name="ps_ot")
                    nc.tensor.transpose(ps_ot[pb:pb + 64, h2, :], o_sb[:], ident[:])
                    if h2 == 0:
                        nc.scalar.copy(out_t[pb:pb + 64, g, i * P:(i + 1) * P],
                                       ps_ot[pb:pb + 64, h2, :])
                    else:
                        nc.vector.tensor_copy(
                            out_t[pb:pb + 64, g, i * P:(i + 1) * P],
                            ps_ot[pb:pb + 64, h2, :])

    pers_cm.__exit__(None, None, None)  # frees v_sb

    # ---------------- Phase C: out-proj, LN2, MLP ------------------------
    with tc.tile_pool(name="wpoolC", bufs=1) as wpoolC, \
         tc.tile_pool(name="ln_sbC", bufs=1) as ln_sbC, \
         tc.tile_pool(name="apool", bufs=1) as apool, \
         tc.tile_pool(name="hpool", bufs=1) as hpool, \
         tc.tile_pool(name="opool", bufs=2) as opool, \
         tc.tile_pool(name="psC", bufs=3, space="PSUM") as psC, \
         tc.tile_pool(name="psC_st", bufs=1, space="PSUM") as psC_st:
        h_t = hpool.tile([P, NCT, TOWN], FR)
        h2f = hpool.tile([P, NCT, TOWN], FR)
        h2b = hpool.tile([P, NCT, TOWN], BF)
        for m in range(NCT):
            ps = psC.tile([P, 512], FP, tag="psC", bufs=3)
            for k in range(NCT):
                nc.tensor.matmul(ps[:], wo_sb[:, m, k, :], out_t[:, k, :],
                                 start=(k == 0), stop=(k == NCT - 1))
            nc.vector.tensor_tensor(h_t[:, m], ps[:], xn_own[:, m], op=OP.add)

        _ln_chunk(nc, ln_sbC, psC_st, h_t, h2f, slice(0, 512),
                  g2c, b2c, onesP, onesPb, eps_t, nb=1)
        for m in range(NCT):
            if m % 2 == 0:
                nc.vector.tensor_copy(h2b[:, m], h2f[:, m])
            else:
                nc.scalar.copy(h2b[:, m], h2f[:, m].bitcast(FP))

        a_t = apool.tile([P, 32, TOWN], BF)    # 4 MB
        for m in range(32):
            w1_t = wpoolC.tile([P, NCT, P], BF, tag="w1t", bufs=4)
            w1_eng = nc.scalar if m % 2 == 0 else nc.sync
            w1_eng.dma_start(w1_t[:], w1[m].rearrange("k p m -> p k m"))
            w1_sl = w1_t[:]
            ps = psC.tile([P, 512], FP, tag="psC", bufs=3)
            for k in range(NCT):
                nc.tensor.matmul(ps[:], w1_sl[:, k], h2b[:, k, :],
                                 start=(k == 0), stop=(k == NCT - 1))
            if m % 2 == 0:
                nc.scalar.activation(a_t[:, m], ps[:], AF.Relu)
            else:
                nc.vector.tensor_scalar_max(a_t[:, m], ps[:], 0.0)

        for m in range(NCT):
            w2_t = wpoolC.tile([P, 32, P], BF, tag="w2t", bufs=2)
            w2_eng = nc.scalar if m % 2 == 0 else nc.sync
            w2_eng.dma_start(w2_t[:], w2[m].rearrange("k p m -> p k m"))
            ps = psC.tile([P, 512], FP, tag="psC", bufs=3)
            for k in range(32):
                nc.tensor.matmul(ps[:], w2_t[:, k], a_t[:, k, :],
                                 start=(k == 0), stop=(k == 31))
            o_m = opool.tile([P, 512], FP, tag="om")
            nc.vector.tensor_tensor(o_m[:], ps[:], h2f[:, m], op=OP.add)
            nc.sync.dma_start(outT[m * P:(m + 1) * P, :], o_m[:])

    persB_cm.__exit__(None, None, None)
    persX_cm.__exit__(None, None, None)
    const_cm.__exit__(None, None, None)


# ---------------------------------------------------------------------------
# Public entry point
# ---------------------------------------------------------------------------
_cache = {}


def _get_nc():
    if "nc" not in _cache:
        _apply_tile_patch()
        nc = bass.Bass("TRN2", target_bir_lowering=False, debug=False,
                       num_devices=8)
        _build(nc)
        _cache["nc"] = nc
    return _cache["nc"]


def run(inputs, trace=False):
    x = np.asarray(inputs["x"], np.float32)
    Wk = np.asarray(inputs["Wk"], np.float32)
    Wq = np.asarray(inputs["Wq"], np.float32)
    Wv = np.asarray(inputs["Wv"], np.float32)
    Wo = np.asarray(inputs["Wo"], np.float32)
    W1 = np.asarray(inputs["W1"], np.float32)
    W2 = np.asarray(inputs["W2"], np.float32)
    g1 = np.asarray(inputs["g1"], np.float32)
    b1 = np.asarray(inputs["b1"], np.float32)
    g2 = np.asarray(inputs["g2"], np.float32)
    b2 = np.asarray(inputs["b2"], np.float32)

    bf = ml_dtypes.bfloat16
    wq_t = _r12(_lhsT_tiles(Wq, NCT, NCT))
    wk_t = _r12(_lhsT_tiles(Wk, NCT, NCT))
    wv_t = _r12(_lhsT_tiles(Wv, NCT, NCT))
    wo_t = _lhsT_tiles(Wo, NCT, NCT).astype(bf)
    w1_t = _lhsT_tiles(W1, NCT, 32).astype(bf)
    w2_t = _lhsT_tiles(W2, 32, NCT).astype(bf)
    gbh = np.stack(
        [g1.reshape(NCT, P).T, b1.reshape(NCT, P).T,
         g2.reshape(NCT, P).T, b2.reshape(NCT, P).T], axis=-1
    ).astype(np.float32)  # [P, NCT, 4]

    in_maps = []
    own_tokens_by_core = []
    for c in range(8):
        b = c // 4
        j = c % 4
        tiles = [j + 4 * i for i in range(NQT)]
        toks = np.concatenate([np.arange(t * P, (t + 1) * P) for t in tiles])
        own_tokens_by_core.append((b, toks))
        xT_full = _r12(np.ascontiguousarray(x[b].T))
        xT_own = _r12(np.ascontiguousarray(x[b][toks].T))
        mask = np.zeros((NQT, P, 512), np.float32)
        for i in range(NQT):
            t0 = (j + 4 * i) * P
            Ei = (i + 1) * 512
            cols = (Ei - 512) + np.arange(512)
            rows = t0 + np.arange(P)
            mask[i] = np.where(cols[None, :] <= rows[:, None], 0.0, -1.0e30)
        in_maps.append({
            "xT": xT_full, "xTo": xT_own,
            "wq": wq_t, "wk": wk_t, "wv": wv_t, "wo": wo_t,
            "w1": w1_t, "w2": w2_t, "gb": gbh, "msk": mask,
        })

    nc = _get_nc()
    res = run_bass_kernel_spmd(nc, in_maps, core_ids=list(range(8)),
                               trace=trace)

    out = np.empty((B, T, C), np.float32)
    for c in range(8):
        b, toks = own_tokens_by_core[c]
        out[b, toks, :] = res.results[c]["outT"].T
    return out, res


def kernel(**inputs):
    out, _ = run(inputs, trace=False)
    return out
